# revision 14
# baseline (speedup 1.0000x reference)
"""Distributed Trainium2 kernel for the Koopman-operator problem.

Math (from the reference):
    X  = x.reshape(64, T)                 # T = 524288, pure row-major view
    M  = L @ L.T                          # 128x128;  M11, M21, M22 are 64x64 blocks
    B  = 2*(M11 + M22 + R - R.T)          # (eps*I is ~1e-8, negligible vs O(30) entries)
    A  = inv(B) @ M21
    out = (A @ X).reshape(-1, 64)

Distribution: column-shard X across 8 cores (65536 cols each) -- fully
data-parallel, zero collectives.  The tiny 64x64 operator A is parameter
preprocessing (O(n^3) vs O(n^2 T) streaming) and is computed once on the
host in float64; the device kernel is a pure bandwidth-bound stream:
out_shard = blockdiag(A,A) @ x_shard.

Per core the shard is pre-stacked on host as (128, 32768): rows 0:64 hold
the first 32768 columns, rows 64:128 the next 32768.  The stationary
matrix is the block-diagonal [[A^T, 0], [0, A^T]] (128x128), which doubles
PE utilization (K=128, M=128 instead of 64).

Bandwidth tricks (the target regime is the HBM ridge):
  * x and out travel as bfloat16 (f32 PSUM accumulation).  Halves HBM
    traffic; measured end-to-end rel err ~3e-3 vs the f32 reference.
  * Input DMAs issue from the SP (sync) HWDGE queue, output DMAs from the
    Activation (scalar) HWDGE queue.  One shared queue serializes loads
    behind stores that wait on compute (head-of-line blocking was the
    dominant stall in the single-queue version).
  * All 8 input chunks are in flight from t=0 (xin pool holds the whole
    8 MiB shard), so loads run at full rate while compute streams behind.
"""

import os
import sys

import numpy as np

for _p in ("/opt/trn_rl_repo", "/root/.axon_site/_ro/trn_rl_repo"):
    if _p not in sys.path and os.path.isdir(_p):
        sys.path.append(_p)

import ml_dtypes

import concourse.bass as bass
import concourse.mybir as mybir
from concourse import bacc
from concourse.bass_utils import run_bass_kernel_spmd

from concourse.tile import TileContext

F32 = mybir.dt.float32
BF16 = mybir.dt.bfloat16
BF16_NP = ml_dtypes.bfloat16

N = 64                   # state dim
N_CORES = 8
T_FULL = 524288          # columns of the reshaped X
T_CORE = T_FULL // N_CORES       # 65536 columns per core
T_HALF = T_CORE // 2             # 32768 -> free dim of the (128, .) shard

MM_COLS = 512            # matmul moving free dim (one PSUM bank, f32)

# Every dma_start expands to 128 descriptors (one per partition).  Read
# descriptors cost ~300ns, write descriptors ~600ns each on the 16 DMA
# engines *independent of size* up to >=16 KiB, so writes need 16 KiB
# descriptors (8192 cols) to reach full bandwidth.  Reads stay finer for
# pipeline granularity, with small leading chunks for an early start.
IN_CHUNKS = [2048, 2048, 4096] + [4096] * 6
N_IN_ON_ACT = 3          # leading chunks ride the Activation queue, whose
                         # DGE starts ~6us before the SP queue's
OUT_CHUNKS = [8192] * 4


def build_kernel(t_half=T_HALF):
    assert sum(IN_CHUNKS) == t_half and sum(OUT_CHUNKS) == t_half
    nc = bacc.Bacc()

    x_ext = nc.declare_dram_parameter("x", [128, t_half], BF16, isOutput=False)
    at_ext = nc.declare_dram_parameter("AT128", [128, 128], BF16, isOutput=False)
    out_ext = nc.declare_dram_parameter("out", [128, t_half], BF16, isOutput=True)

    with TileContext(nc) as tc:
        with (
            tc.tile_pool(name="const", bufs=1) as cpool,
            tc.tile_pool(name="xin", bufs=len(IN_CHUNKS)) as xpool,
            tc.tile_pool(name="yout", bufs=3) as opool,
            tc.tile_pool(name="mm_ps", bufs=8, space="PSUM") as mps,
        ):
            # stationary blockdiag(A^T, A^T) rides the (initially idle)
            # Activation queue so the SP queue starts on x immediately
            at_sb = cpool.tile([128, 128], BF16)
            nc.scalar.dma_start(out=at_sb[:], in_=at_ext[:, :])

            # prefetch the whole shard: loads only ever wait on the queue
            # itself, never on compute
            tile_src = {}  # 512-col tile index -> (xin tile, col offset)
            base = 0
            for ci, cols in enumerate(IN_CHUNKS):
                xin = xpool.tile([128, cols], BF16, tag="xin")
                eng = nc.scalar if ci < N_IN_ON_ACT else nc.sync
                eng.dma_start(out=xin[:], in_=x_ext[:, base : base + cols])
                for j in range(cols // MM_COLS):
                    tile_src[base // MM_COLS + j] = (xin, j * MM_COLS)
                base += cols

            # stream: matmul 512-col tiles into PSUM, cast-copy to bf16
            # SBUF (alternating DVE/Activation so neither paces the PE),
            # store each chunk from the Activation queue
            obase = 0
            for cols in OUT_CHUNKS:
                yout = opool.tile([128, cols], BF16, tag="yout", name="yout")
                for j in range(cols // MM_COLS):
                    g = obase // MM_COLS + j
                    xin, xoff = tile_src[g]
                    ps = mps.tile([128, MM_COLS], F32, tag="mm")
                    nc.tensor.matmul(
                        ps[:],
                        lhsT=at_sb[:],
                        rhs=xin[:, xoff : xoff + MM_COLS],
                        start=True,
                        stop=True,
                    )
                    dst = yout[:, j * MM_COLS : (j + 1) * MM_COLS]
                    if g % 2 == 0:
                        nc.vector.tensor_copy(out=dst, in_=ps[:])
                    else:
                        nc.scalar.copy(out=dst, in_=ps[:])
                nc.scalar.dma_start(
                    out=out_ext[:, obase : obase + cols], in_=yout[:]
                )
                obase += cols

    return nc


_NC_CACHE = {}
LAST_PROFILE = None


def _get_nc(t_half=T_HALF):
    if t_half not in _NC_CACHE:
        nc = build_kernel(t_half)
        nc.finalize()  # Bacc: reg alloc + event-semaphore wait splitting
        _NC_CACHE[t_half] = nc
    return _NC_CACHE[t_half]


def _ensure_ntff_hook():
    """The agent image's `antenv` lacks the `axon_hooks` shim that
    `trn_agent_boot` uses to register the NTFF profiling hook (boot
    degrades silently).  Provide the shim and register the hook so
    run_bass_kernel_spmd(trace=True) can capture neuron-profile data."""
    import types

    try:
        from antenv.axon_hooks import get_axon_ntff_profile_hook  # noqa: F401
        return True
    except ImportError:
        pass
    try:
        import antenv
        from trn_agent_boot.trn_boot import _ntff_profile_via_ctypes

        mod = types.ModuleType("antenv.axon_hooks")
        _store = {"h": None}
        mod.set_axon_ntff_profile_hook = lambda h: _store.__setitem__("h", h)
        mod.get_axon_ntff_profile_hook = lambda: _store["h"]
        sys.modules["antenv.axon_hooks"] = mod
        antenv.axon_hooks = mod
        hook = _ntff_profile_via_ctypes("/opt/axon/libaxon_pjrt.so")
        mod.set_axon_ntff_profile_hook(hook)
        return hook is not None
    except Exception as e:  # degrade to no-trace
        print(f"kernel.py: NTFF hook setup failed ({type(e).__name__}: {e})")
        return False


def kernel(x, L, R):
    global LAST_PROFILE
    x = np.ascontiguousarray(np.asarray(x, dtype=np.float32))
    L = np.asarray(L, dtype=np.float32)
    R = np.asarray(R, dtype=np.float32)
    assert x.shape == (T_FULL, N), x.shape

    # tiny operator, host float64: A = inv(2*(M11+M22+R-R^T)) @ M21
    M = L.astype(np.float64) @ L.T.astype(np.float64)
    M += 1e-8 * np.eye(2 * N)
    B = 2.0 * (M[:N, :N] + M[N:, N:] + R.astype(np.float64) - R.T.astype(np.float64))
    A = np.linalg.solve(B, M[:N, N:])
    at128 = np.zeros((128, 128), dtype=BF16_NP)
    at128[:N, :N] = A.T.astype(BF16_NP)
    at128[N:, N:] = at128[:N, :N]

    X = x.reshape(N, T_FULL).astype(BF16_NP)  # round-to-nearest-even
    in_maps = []
    for c in range(N_CORES):
        shard = np.empty((128, T_HALF), dtype=BF16_NP)
        base = c * T_CORE
        shard[:N] = X[:, base : base + T_HALF]
        shard[N:] = X[:, base + T_HALF : base + T_CORE]
        in_maps.append({"x": shard, "AT128": at128})

    nc = _get_nc()
    trace = os.environ.get("KERNEL_TRACE", "0") == "1"
    if trace:
        trace = _ensure_ntff_hook()
    try:
        res = run_bass_kernel_spmd(
            nc, in_maps, core_ids=list(range(N_CORES)), trace=trace
        )
    except Exception:
        if not trace:
            raise
        print("kernel.py: traced run failed; retrying without trace")
        res = run_bass_kernel_spmd(
            nc, in_maps, core_ids=list(range(N_CORES)), trace=False
        )
    LAST_PROFILE = res

    Y = np.empty((N, T_FULL), dtype=np.float32)
    for c in range(N_CORES):
        o = np.asarray(res.results[c]["out"]).astype(np.float32)
        base = c * T_CORE
        Y[:, base : base + T_HALF] = o[:N]
        Y[:, base + T_HALF : base + T_CORE] = o[N:]
    return Y.reshape(T_FULL, N)


# revision 17
# speedup vs baseline: 1.0530x; 1.0530x over previous
"""Distributed Trainium2 kernel for the Koopman-operator problem.

Math (from the reference):
    X  = x.reshape(64, T)                 # T = 524288, pure row-major view
    M  = L @ L.T                          # 128x128;  M11, M21, M22 are 64x64 blocks
    B  = 2*(M11 + M22 + R - R.T)          # (eps*I is ~1e-8, negligible vs O(30) entries)
    A  = inv(B) @ M21
    out = (A @ X).reshape(-1, 64)

Distribution: column-shard X across 8 cores (65536 cols each) -- fully
data-parallel, zero collectives.  The tiny 64x64 operator A is parameter
preprocessing (O(n^3) vs O(n^2 T) streaming) and is computed once on the
host in float64; the device kernel is a pure bandwidth-bound stream:
out_shard = blockdiag(A,A) @ x_shard.

Per core the shard is pre-stacked on host as (128, 32768): rows 0:64 hold
the first 32768 columns, rows 64:128 the next 32768.  The stationary
matrix is the block-diagonal [[A^T, 0], [0, A^T]] (128x128), which doubles
PE utilization (K=128, M=128 instead of 64).

Bandwidth tricks (the target regime is the HBM ridge):
  * x and out travel as bfloat16 (f32 PSUM accumulation).  Halves HBM
    traffic; measured end-to-end rel err ~3e-3 vs the f32 reference.
  * Input DMAs issue from the SP (sync) HWDGE queue, output DMAs from the
    Activation (scalar) HWDGE queue.  One shared queue serializes loads
    behind stores that wait on compute (head-of-line blocking was the
    dominant stall in the single-queue version).
  * All 8 input chunks are in flight from t=0 (xin pool holds the whole
    8 MiB shard), so loads run at full rate while compute streams behind.
"""

import os
import sys

import numpy as np

for _p in ("/opt/trn_rl_repo", "/root/.axon_site/_ro/trn_rl_repo"):
    if _p not in sys.path and os.path.isdir(_p):
        sys.path.append(_p)

import ml_dtypes

import concourse.bass as bass
import concourse.mybir as mybir
from concourse import bacc
from concourse.bass_utils import run_bass_kernel_spmd

from concourse.tile import TileContext

F32 = mybir.dt.float32
BF16 = mybir.dt.bfloat16
BF16_NP = ml_dtypes.bfloat16

N = 64                   # state dim
N_CORES = 8
T_FULL = 524288          # columns of the reshaped X
T_CORE = T_FULL // N_CORES       # 65536 columns per core
T_HALF = T_CORE // 2             # 32768 -> free dim of the (128, .) shard

MM_COLS = 512            # matmul moving free dim (one PSUM bank, f32)

# Per-chunk DRAM tensors, each a contiguous (128 x cols) block.  A DMA
# whose DRAM side is fully contiguous coalesces into a handful of large
# multi-partition descriptors (like the tiny AT128 load) instead of 128
# per-partition ones, sidestepping the ~300-600ns-per-descriptor wall.
IN_COLS = 4096           # input chunk: 1 MiB contiguous block
OUT_COLS = 4096          # output chunk: 1 MiB contiguous block
N_IN = T_HALF // IN_COLS
N_OUT = T_HALF // OUT_COLS


def build_kernel(t_half=T_HALF):
    nc = bacc.Bacc()

    x_exts = [
        nc.declare_dram_parameter(f"x{c}", [128, IN_COLS], BF16, isOutput=False)
        for c in range(N_IN)
    ]
    at_ext = nc.declare_dram_parameter("AT128", [128, 128], BF16, isOutput=False)
    out_exts = [
        nc.declare_dram_parameter(f"out{c}", [128, OUT_COLS], BF16, isOutput=True)
        for c in range(N_OUT)
    ]

    with TileContext(nc) as tc:
        with (
            tc.tile_pool(name="const", bufs=1) as cpool,
            tc.tile_pool(name="xin", bufs=N_IN) as xpool,
            tc.tile_pool(name="yout", bufs=3) as opool,
            tc.tile_pool(name="mm_ps", bufs=8, space="PSUM") as mps,
        ):
            # stationary blockdiag(A^T, A^T)
            at_sb = cpool.tile([128, 128], BF16)
            nc.sync.dma_start(out=at_sb[:], in_=at_ext[:, :])

            # prefetch the whole shard on the SP queue: loads only ever
            # wait on the queue itself, never on compute
            xins = []
            for c in range(N_IN):
                xin = xpool.tile([128, IN_COLS], BF16, tag="xin")
                nc.sync.dma_start(out=xin[:], in_=x_exts[c][:, :])
                xins.append(xin)

            # stream: matmul 512-col tiles into PSUM, cast-copy to bf16
            # SBUF (alternating DVE/Activation so neither paces the PE),
            # store each chunk from the Activation queue
            for c in range(N_OUT):
                yout = opool.tile([128, OUT_COLS], BF16, tag="yout", name="yout")
                for j in range(OUT_COLS // MM_COLS):
                    g = (c * OUT_COLS + j * MM_COLS)
                    xin = xins[g // IN_COLS]
                    xoff = g % IN_COLS
                    ps = mps.tile([128, MM_COLS], F32, tag="mm")
                    nc.tensor.matmul(
                        ps[:],
                        lhsT=at_sb[:],
                        rhs=xin[:, xoff : xoff + MM_COLS],
                        start=True,
                        stop=True,
                    )
                    dst = yout[:, j * MM_COLS : (j + 1) * MM_COLS]
                    if (g // MM_COLS) % 2 == 0:
                        nc.vector.tensor_copy(out=dst, in_=ps[:])
                    else:
                        nc.scalar.copy(out=dst, in_=ps[:])
                nc.scalar.dma_start(out=out_exts[c][:, :], in_=yout[:])

    return nc


_NC_CACHE = {}
LAST_PROFILE = None


def _get_nc(t_half=T_HALF):
    if t_half not in _NC_CACHE:
        nc = build_kernel(t_half)
        nc.finalize()  # Bacc: reg alloc + event-semaphore wait splitting
        _NC_CACHE[t_half] = nc
    return _NC_CACHE[t_half]


def _ensure_ntff_hook():
    """The agent image's `antenv` lacks the `axon_hooks` shim that
    `trn_agent_boot` uses to register the NTFF profiling hook (boot
    degrades silently).  Provide the shim and register the hook so
    run_bass_kernel_spmd(trace=True) can capture neuron-profile data."""
    import types

    try:
        from antenv.axon_hooks import get_axon_ntff_profile_hook  # noqa: F401
        return True
    except ImportError:
        pass
    try:
        import antenv
        from trn_agent_boot.trn_boot import _ntff_profile_via_ctypes

        mod = types.ModuleType("antenv.axon_hooks")
        _store = {"h": None}
        mod.set_axon_ntff_profile_hook = lambda h: _store.__setitem__("h", h)
        mod.get_axon_ntff_profile_hook = lambda: _store["h"]
        sys.modules["antenv.axon_hooks"] = mod
        antenv.axon_hooks = mod
        hook = _ntff_profile_via_ctypes("/opt/axon/libaxon_pjrt.so")
        mod.set_axon_ntff_profile_hook(hook)
        return hook is not None
    except Exception as e:  # degrade to no-trace
        print(f"kernel.py: NTFF hook setup failed ({type(e).__name__}: {e})")
        return False


def kernel(x, L, R):
    global LAST_PROFILE
    x = np.ascontiguousarray(np.asarray(x, dtype=np.float32))
    L = np.asarray(L, dtype=np.float32)
    R = np.asarray(R, dtype=np.float32)
    assert x.shape == (T_FULL, N), x.shape

    # tiny operator, host float64: A = inv(2*(M11+M22+R-R^T)) @ M21
    M = L.astype(np.float64) @ L.T.astype(np.float64)
    M += 1e-8 * np.eye(2 * N)
    B = 2.0 * (M[:N, :N] + M[N:, N:] + R.astype(np.float64) - R.T.astype(np.float64))
    A = np.linalg.solve(B, M[:N, N:])
    at128 = np.zeros((128, 128), dtype=BF16_NP)
    at128[:N, :N] = A.T.astype(BF16_NP)
    at128[N:, N:] = at128[:N, :N]

    X = x.reshape(N, T_FULL).astype(BF16_NP)  # round-to-nearest-even
    in_maps = []
    for c in range(N_CORES):
        base = c * T_CORE
        m = {"AT128": at128}
        for k in range(N_IN):
            blk = np.empty((128, IN_COLS), dtype=BF16_NP)
            cb = base + k * IN_COLS
            blk[:N] = X[:, cb : cb + IN_COLS]
            blk[N:] = X[:, cb + T_HALF : cb + T_HALF + IN_COLS]
            m[f"x{k}"] = blk
        in_maps.append(m)

    nc = _get_nc()
    trace = os.environ.get("KERNEL_TRACE", "0") == "1"
    if trace:
        trace = _ensure_ntff_hook()
    try:
        res = run_bass_kernel_spmd(
            nc, in_maps, core_ids=list(range(N_CORES)), trace=trace
        )
    except Exception:
        if not trace:
            raise
        print("kernel.py: traced run failed; retrying without trace")
        res = run_bass_kernel_spmd(
            nc, in_maps, core_ids=list(range(N_CORES)), trace=False
        )
    LAST_PROFILE = res

    Y = np.empty((N, T_FULL), dtype=np.float32)
    for c in range(N_CORES):
        base = c * T_CORE
        for k in range(N_OUT):
            o = np.asarray(res.results[c][f"out{k}"]).astype(np.float32)
            cb = base + k * OUT_COLS
            Y[:, cb : cb + OUT_COLS] = o[:N]
            Y[:, cb + T_HALF : cb + T_HALF + OUT_COLS] = o[N:]
    return Y.reshape(T_FULL, N)
